# revision 17
# baseline (speedup 1.0000x reference)
"""Trainium2 Bass kernel for nn_BatchedModelManifoldGeodesicFlow.

Math (validated vs reference):
  G = J^T J + eps*I is symmetric => the Christoffel contraction collapses:
    einsum('bijk,bj,bk->bi', Gamma, v, v) = 0.5 * einsum('bijk,bj,bk->bi', dG, v, v)
  With f = tanh(x@W1+b1)@W2+b2 (J = W2^T diag(d1) W1^T, per-output Hessians
  H_o = W1 diag(W2[:,o]*d2) W1^T where d1 = 1-s^2, d2 = -2*s*d1, s = tanh(z)):

    T_i := sum_jk dG[i,j,k] v_j v_k = (W1 @ g)_i,
       g = d2*p*(S@(d1*p)) + d1*(S@(d2*p*p)),  p = W1^T v,  S = W2@W2^T
    ||dG||_F^2 = 2*(<G1,G2> + sum_{o,o'} Y[:,(o,o')].Y[:,(o',o)])
       E = d1[:,None]*W2, C = d2[:,None]*W2, K = W1^T W1, K2 = K*K
       F = K@E, G1 = E^T F, G2 = C^T (K2@C), Y[:, o*O+o'] = W1@(C[:,o']*F[:,o])
    a = -0.5*T/((||dG||_F+1e-6)*(||v||+1e-6));  out = concat(v, a - 0.1*dev)

Sharding: pure batch parallelism, B=32 over 8 cores (4 samples/core),
params replicated. Feature-major on-chip layout throughout ([d or h
partitions, batch columns]); host pre-transposes inputs and parameter
derivatives (W1T, W2T, K, K2) so the kernel has no PE transposes; the
output store uses a transposed DRAM access pattern.
"""

import os
import sys

if "/opt/trn_rl_repo" not in sys.path:
    sys.path.insert(0, "/opt/trn_rl_repo")

import numpy as np

B, D, H, O = 32, 128, 256, 10
NCORES = 8
BC = B // NCORES  # 4 samples per core
OO = O * O  # 100

_PROGRAM = None


def _build_program():
    import concourse.bass as bass
    import concourse.bacc as bacc
    import concourse.tile as tile
    from concourse import mybir

    f32 = mybir.dt.float32
    mult = mybir.AluOpType.mult
    add = mybir.AluOpType.add
    AF = mybir.ActivationFunctionType

    nc = bacc.Bacc(None)
    # xc: cols 0:4 dev, 4:8 x1, 8:12 x0, 12:16 v (feature-major), col 16 t
    xc_d = nc.declare_dram_parameter("xc", [D, 4 * BC + 1], f32, isOutput=False)
    # aux: cols 0:2 b1 (h-chunked), 2:22 W2 (cols hc*10+o)
    aux_d = nc.declare_dram_parameter("aux", [D, 22], f32, isOutput=False)
    # wp: cols 0:256 W1, cols 256+hc*128 W1T chunk hc
    wp_d = nc.declare_dram_parameter("wp", [D, H + H], f32, isOutput=False)
    # kq: K = W1^T W1 row-chunks [128, 256] each; k2q: K*K likewise
    kq_d = nc.declare_dram_parameter("kq", [D, 2 * H], f32, isOutput=False)
    k2q_d = nc.declare_dram_parameter("k2q", [D, 2 * H], f32, isOutput=False)
    # w2t: W2^T [O, H]
    w2t_d = nc.declare_dram_parameter("w2t", [O, H], f32, isOutput=False)
    acc_d = nc.declare_dram_parameter("acc", [BC, D], f32, isOutput=True)

    with tile.TileContext(nc) as tc:
        with (
            tc.tile_pool(name="const", bufs=1) as const,
            tc.tile_pool(name="work", bufs=1) as work,
            tc.tile_pool(name="ps", bufs=2, space="PSUM") as ps,
        ):
            # ---------------- parallel input DMAs (one per engine queue) ----------------
            xc_sb = const.tile([D, 4 * BC + 1], f32)
            nc.sync.dma_start(out=xc_sb, in_=xc_d[:])
            wp_sb = const.tile([D, 2 * H], f32)
            nc.scalar.dma_start(out=wp_sb, in_=wp_d[:])
            kq_sb = const.tile([D, 2 * H], f32)
            nc.scalar.dma_start(out=kq_sb, in_=kq_d[:])
            aux_sb = const.tile([D, 22], f32)
            nc.gpsimd.dma_start(out=aux_sb, in_=aux_d[:])
            w2t_sb = const.tile([O, H], f32)
            nc.sync.dma_start(out=w2t_sb, in_=w2t_d[:])
            k2q_sb = const.tile([D, 2 * H], f32)
            nc.gpsimd.dma_start(out=k2q_sb, in_=k2q_d[:])

            b1_sb = aux_sb[:, 0:2]
            w2_sb = aux_sb[:, 2:22]
            w1_sb = wp_sb[:, 0:H]
            w1t = [wp_sb[:, H + hc * 128 : H + (hc + 1) * 128] for hc in range(2)]
            k_sb = [kq_sb[:, kc * H : (kc + 1) * H] for kc in range(2)]
            k2_sb = [k2q_sb[:, kc * H : (kc + 1) * H] for kc in range(2)]
            tt = xc_sb[:, 4 * BC : 4 * BC + 1]
            devt = xc_sb[:, 0:BC]
            x1t = xc_sb[:, BC : 2 * BC]
            x0t = xc_sb[:, 2 * BC : 3 * BC]
            vt = xc_sb[:, 3 * BC : 4 * BC]

            ones_c = const.tile([128, 1], f32)
            nc.vector.memset(ones_c, 1.0)
            twos_c = const.tile([128, 1], f32)
            nc.vector.memset(twos_c, 2.0)
            ones_r = const.tile([1, 128], f32)
            nc.vector.memset(ones_r, 1.0)

            # ---------------- x = x0 + t*(x1-x0) + 4t(1-t)*dev ----------------
            # wrow = 4t(1-t) on DVE; element chain on Pool via stride-0 views
            omt = work.tile([128, 1], f32)
            nc.vector.tensor_scalar(omt, tt, -1.0, 1.0, mult, add)
            wrow = work.tile([128, 1], f32)
            nc.vector.scalar_tensor_tensor(wrow, tt, 4.0, omt, mult, mult)

            def col_bcast(ap1, n):
                return bass.AP(
                    tensor=ap1.tensor, offset=ap1.offset, ap=[ap1.ap[0], [0, n]]
                )

            dx = work.tile([128, BC], f32)
            nc.gpsimd.tensor_sub(dx, x1t, x0t)
            txm = work.tile([128, BC], f32)
            nc.gpsimd.tensor_tensor(txm, dx, col_bcast(tt, BC), mult)
            xa = work.tile([128, BC], f32)
            nc.gpsimd.tensor_add(xa, txm, x0t)
            wdev = work.tile([128, BC], f32)
            nc.gpsimd.tensor_tensor(wdev, devt, col_bcast(wrow, BC), mult)
            # xf overwrites the x0 slot so [xf | v] is contiguous for the z/p matmul
            xf = xc_sb[:, 2 * BC : 3 * BC]
            nc.gpsimd.tensor_add(xf, wdev, xa)
            zp_rhs = xc_sb[:, 2 * BC : 4 * BC]

            devneg = work.tile([128, BC], f32)
            nc.vector.tensor_scalar_mul(devneg, devt, -0.1)
            vsq = work.tile([128, BC], f32)
            nc.gpsimd.tensor_mul(vsq, vt, vt)

            # ---------------- z = W1^T x (+b1, tanh), p = W1^T v ----------------
            s_act = work.tile([128, 2 * BC], f32)  # tanh(z), cols hc*BC+b
            ps_zp = []
            for hc in range(2):
                ps_z = ps.tile([128, 2 * BC], f32, tag="ps", bufs=2)
                nc.tensor.matmul(ps_z, w1_sb[:, hc * 128 : (hc + 1) * 128], zp_rhs, start=True, stop=True)
                nc.scalar.activation(
                    s_act[:, hc * BC : (hc + 1) * BC], ps_z[:, 0:BC], AF.Tanh, bias=b1_sb[:, hc : hc + 1]
                )
                ps_zp.append(ps_z)

            # ---------------- d1 = 1-s^2 (DVE), d2' = s*d1 (Pool) ----------------
            # true d2 = -2*d2'; T and ||dG|| scale uniformly, compensated by the
            # 0.25 scale inside the final sqrt and the positive output sign.
            d1 = work.tile([128, 2 * BC], f32)
            nc.vector.tensor_mul(d1, s_act, s_act)
            nc.vector.tensor_scalar(d1, d1, -1.0, 1.0, mult, add)
            d2 = work.tile([128, 2 * BC], f32)
            nc.gpsimd.tensor_mul(d2, s_act, d1)

            # ---------------- E = d1*W2, C = d2*W2 (Pool, broadcast views) ----------------
            e_all = []
            c_all = []
            for hc in range(2):
                e_t = work.tile([128, BC * O], f32, tag=f"e{hc}")
                c_t = work.tile([128, BC * O], f32, tag=f"c{hc}")
                w2_blk = w2_sb[:, hc * O : (hc + 1) * O]
                w2_view = bass.AP(
                    tensor=w2_blk.tensor, offset=w2_blk.offset,
                    ap=[w2_blk.ap[0], [0, BC], list(w2_blk.ap[1])],
                )
                d1_blk = d1[:, hc * BC : (hc + 1) * BC]
                d1_view = bass.AP(
                    tensor=d1_blk.tensor, offset=d1_blk.offset,
                    ap=[d1_blk.ap[0], list(d1_blk.ap[1]), [0, O]],
                )
                d2_blk = d2[:, hc * BC : (hc + 1) * BC]
                d2_view = bass.AP(
                    tensor=d2_blk.tensor, offset=d2_blk.offset,
                    ap=[d2_blk.ap[0], list(d2_blk.ap[1]), [0, O]],
                )
                nc.gpsimd.tensor_tensor(
                    e_t[:].rearrange("p (b o) -> p b o", b=BC), w2_view, d1_view, mult
                )
                nc.gpsimd.tensor_tensor(
                    c_t[:].rearrange("p (b o) -> p b o", b=BC), w2_view, d2_view, mult
                )
                e_all.append(e_t)
                c_all.append(c_t)

            # ---------------- F = K@E, KC = K2@C ----------------
            f_sb = work.tile([128, 2 * BC * O], f32)  # cols mc*40 + (b*10+o)
            kc_sb = work.tile([128, 2 * BC * O], f32)
            ps_fl = []
            ps_kcl = []
            for mc in range(2):
                ps_f = ps.tile([128, BC * O], f32, tag="ps", bufs=2)
                for kc in range(2):
                    nc.tensor.matmul(
                        ps_f, k_sb[kc][:, mc * 128 : (mc + 1) * 128], e_all[kc], start=(kc == 0), stop=(kc == 1)
                    )
                ps_fl.append(ps_f)
                nc.vector.tensor_copy(f_sb[:, mc * BC * O : (mc + 1) * BC * O], ps_f)
            for mc in range(2):
                ps_kc = ps.tile([128, BC * O], f32, tag="ps", bufs=2)
                for kc in range(2):
                    nc.tensor.matmul(
                        ps_kc, k2_sb[kc][:, mc * 128 : (mc + 1) * 128], c_all[kc], start=(kc == 0), stop=(kc == 1)
                    )
                ps_kcl.append(ps_kc)
                nc.scalar.copy(kc_sb[:, mc * BC * O : (mc + 1) * BC * O], ps_kc)

            # trigger the sqrt act-table load early (Copy works in every table)
            sqrt_dummy = work.tile([1, 1], f32)
            nc.scalar.activation(sqrt_dummy, ones_r[:, 0:1], AF.Sqrt)

            # ---------------- q = S @ [d1*p | d2*p*p] (p read from PSUM) ----------------
            d2p = work.tile([128, 2 * BC], f32)
            qrhs = work.tile([128, 4 * BC], f32)  # cols: [d1p (2*BC) | d2pp (2*BC)]
            for hc in range(2):
                hsl = slice(hc * BC, (hc + 1) * BC)
                p_blk = ps_zp[hc][:, BC : 2 * BC]
                nc.vector.tensor_mul(d2p[:, hsl], d2[:, hsl], p_blk)
                nc.vector.tensor_mul(qrhs[:, hc * BC : hc * BC + BC], d1[:, hsl], p_blk)
                nc.vector.tensor_mul(
                    qrhs[:, 2 * BC + hc * BC : 2 * BC + hc * BC + BC], d2p[:, hsl], p_blk
                )

            qv = qrhs[:].rearrange("p (g c b) -> p c g b", g=2, c=2, b=BC)
            ps_u = ps.tile([O, 2 * BC], f32, tag="q", bufs=3)
            for kc in range(2):
                nc.tensor.matmul(
                    ps_u, w2_sb[:, kc * O : (kc + 1) * O], qv[:, kc], start=(kc == 0), stop=(kc == 1)
                )
            u_sb = work.tile([O, 2 * BC], f32)
            nc.vector.tensor_copy(u_sb, ps_u)
            ps_q = []
            for mc in range(2):
                ps_qm = ps.tile([128, 2 * BC], f32, tag="q", bufs=3)
                nc.tensor.matmul(
                    ps_qm, w2t_sb[:, mc * 128 : (mc + 1) * 128], u_sb, start=True, stop=True
                )
                ps_q.append(ps_qm)

            # ---------------- g = d2*p*q1 + d1*q2 (Pool) ; T = W1 @ g ----------------
            g_t = work.tile([128, 2 * BC], f32)
            tmp_dq = work.tile([128, 2 * BC], f32)
            for hc in range(2):
                hsl = slice(hc * BC, (hc + 1) * BC)
                nc.vector.tensor_mul(g_t[:, hsl], d2p[:, hsl], ps_q[hc][:, 0:BC])
                nc.vector.tensor_mul(tmp_dq[:, hsl], d1[:, hsl], ps_q[hc][:, BC : 2 * BC])
            nc.vector.tensor_add(g_t, g_t, tmp_dq)
            ps_T = ps.tile([128, BC], f32, tag="T", bufs=1)
            for hc in range(2):
                nc.tensor.matmul(ps_T, w1t[hc], g_t[:, hc * BC : (hc + 1) * BC], start=(hc == 0), stop=(hc == 1))
            t_sb = work.tile([128, BC], f32)
            nc.vector.tensor_copy(t_sb, ps_T)

            # ---------------- Mx build (Pool) + Y = W1 @ Mx + termB from PSUM ----------------
            mx = [
                work.tile([128, BC * OO], f32, tag="mx0", name="mx_t0"),
                work.tile([128, BC * OO], f32, tag="mx1", name="mx_t1"),
            ]
            for hc in range(2):
                for b in range(BC):
                    mx_t = mx[hc]
                    c_blk = c_all[hc][:, b * O : (b + 1) * O]
                    f_blk = f_sb[:, hc * BC * O + b * O : hc * BC * O + (b + 1) * O]
                    c_view = bass.AP(
                        tensor=c_blk.tensor, offset=c_blk.offset, ap=[c_blk.ap[0], [0, O], list(c_blk.ap[1])]
                    )
                    f_view = bass.AP(
                        tensor=f_blk.tensor, offset=f_blk.offset, ap=[f_blk.ap[0], list(f_blk.ap[1]), [0, O]]
                    )
                    nc.gpsimd.tensor_tensor(
                        mx_t[:, b * OO : (b + 1) * OO].rearrange("p (a c) -> p a c", a=O),
                        f_view,
                        c_view,
                        mult,
                    )

            half = BC // 2 * OO  # 200 cols per half (samples b=2hf, 2hf+1)
            tb = work.tile([128, BC], f32)
            junk_b = work.tile([128, BC * OO], f32)
            ps_y = [
                ps.tile([128, half], f32, tag="Y", bufs=2, name=f"ps_y{_hf}")
                for _hf in range(2)
            ]
            for hc in range(2):
                for hf in range(2):
                    nc.tensor.matmul(
                        ps_y[hf],
                        w1t[hc],
                        mx[hc][:, hf * half : (hf + 1) * half],
                        start=(hc == 0),
                        stop=(hc == 1),
                    )
            y_sb = work.tile([128, BC * OO], f32)
            for hf in range(2):
                ysl = y_sb[:, hf * half : (hf + 1) * half]
                if hf == 0:
                    nc.scalar.copy(ysl, ps_y[hf])
                else:
                    nc.vector.tensor_copy(ysl, ps_y[hf])
                for bi in range(2):
                    b = hf * 2 + bi
                    blk = y_sb[:, b * OO : (b + 1) * OO]
                    jb = junk_b[:, b * OO : (b + 1) * OO]
                    nc.vector.scalar_tensor_tensor(
                        jb.rearrange("p (a c) -> p a c", a=O),
                        ps_y[hf][:, bi * OO : (bi + 1) * OO].rearrange("p (a c) -> p a c", a=O),
                        1.0,
                        blk.rearrange("p (a c) -> p c a", a=O, c=O),
                        mult,
                        mult,
                        accum_out=tb[:, b : b + 1],
                    )

            # ------------ termA: G1_b = E_b^T F_b, G2_b = C_b^T KC_b ; <G1,G2> ------------
            ta = work.tile([O, BC], f32)
            junk_a = work.tile([O, BC * O], f32)
            ps_g1 = ps.tile([O, BC * O], f32, tag="q", bufs=3)
            ps_g2 = ps.tile([O, BC * O], f32, tag="q", bufs=3)
            for b in range(BC):
                for hc in range(2):
                    nc.tensor.matmul(
                        ps_g1[:, b * O : (b + 1) * O],
                        e_all[hc][:, b * O : (b + 1) * O],
                        f_sb[:, hc * BC * O + b * O : hc * BC * O + (b + 1) * O],
                        start=(hc == 0),
                        stop=(hc == 1),
                        skip_group_check=True,
                    )
                for hc in range(2):
                    nc.tensor.matmul(
                        ps_g2[:, b * O : (b + 1) * O],
                        c_all[hc][:, b * O : (b + 1) * O],
                        kc_sb[:, hc * BC * O + b * O : hc * BC * O + (b + 1) * O],
                        start=(hc == 0),
                        stop=(hc == 1),
                        skip_group_check=True,
                    )
            g2_sb = work.tile([O, BC * O], f32)
            nc.vector.tensor_copy(g2_sb, ps_g2)
            for b in range(BC):
                ja = junk_a[:, b * O : (b + 1) * O]
                nc.vector.scalar_tensor_tensor(
                    ja,
                    ps_g1[:, b * O : (b + 1) * O],
                    1.0,
                    g2_sb[:, b * O : (b + 1) * O],
                    mult,
                    mult,
                    accum_out=ta[:, b : b + 1],
                )

            # ---------------- reductions: nf^2 = 2*(termA+termB), |v|^2 ----------------
            ps_red = ps.tile([1, 2 * BC], f32, tag="q", bufs=3)
            nc.tensor.matmul(ps_red[:, BC : 2 * BC], ones_c, vsq, start=True, stop=True, skip_group_check=True)
            vn_sb = work.tile([1, BC], f32)
            nc.vector.tensor_copy(vn_sb, ps_red[:, BC : 2 * BC])
            nc.tensor.matmul(ps_red[:, 0:BC], twos_c[0:O], ta, start=True, stop=False, skip_group_check=True)
            nc.tensor.matmul(ps_red[:, 0:BC], twos_c, tb, start=False, stop=True, skip_group_check=True)
            # rsc = 1/(nf*|v|) = sqrt(1/(nf^2*|v|^2)) : reciprocal on DVE, sqrt on ACT
            qrow = work.tile([1, BC], f32)
            nc.vector.tensor_mul(qrow, ps_red[:, 0:BC], vn_sb)
            nc.vector.reciprocal(qrow, qrow)
            rsc_row = work.tile([1, BC], f32)
            nc.scalar.activation(rsc_row, qrow, AF.Sqrt, scale=0.25)

            # broadcast rsc over partitions via PE outer product
            ps_rscb = ps.tile([128, BC], f32, tag="q", bufs=3)
            nc.tensor.matmul(ps_rscb, ones_r, rsc_row, start=True, stop=True, skip_group_check=True)

            # out = T*rsc - 0.1*dev (feature-major), stored via transposed DRAM AP
            out_fm = work.tile([128, BC], f32)
            nc.vector.tensor_mul(out_fm, t_sb, ps_rscb)
            nc.vector.tensor_add(out_fm, out_fm, devneg)
            acc_ap = acc_d[:]
            acc_tv = bass.AP(tensor=acc_ap.tensor, offset=acc_ap.offset, ap=[[1, 128], [128, BC]])
            nc.sync.dma_start(out=acc_tv, in_=out_fm)

    nc.finalize()
    return nc


def _get_program():
    global _PROGRAM
    if _PROGRAM is None:
        _PROGRAM = _build_program()
    return _PROGRAM


def make_in_maps(t, state_batch, x0, x1, W1, b1, W2):
    dev = state_batch[:B]
    v = state_batch[B:]
    w1_arr = np.asarray(W1, np.float32)
    aux = np.empty((128, 22), np.float32)
    aux[:, 0:2] = np.asarray(b1, np.float32).reshape(2, 128).T
    aux[:, 2:22] = (
        np.asarray(W2, np.float32).reshape(2, 128, O).transpose(1, 0, 2).reshape(128, 2 * O)
    )
    wp = np.empty((128, 2 * H), np.float32)
    wp[:, 0:H] = w1_arr
    wp[:, H : H + 128] = w1_arr[:, 0:128].T
    wp[:, H + 128 : 2 * H] = w1_arr[:, 128:256].T
    k64 = np.asarray(W1, np.float64)
    K = (k64.T @ k64).astype(np.float32)
    kq = np.ascontiguousarray(np.concatenate([K[0:128, :], K[128:256, :]], axis=1))
    K2 = K * K
    k2q = np.ascontiguousarray(np.concatenate([K2[0:128, :], K2[128:256, :]], axis=1))
    w2t = np.ascontiguousarray(np.asarray(W2, np.float32).T)
    tval = np.float32(np.asarray(t).ravel()[0])
    in_maps = []
    for c in range(NCORES):
        sl = slice(c * BC, (c + 1) * BC)
        xc = np.empty((128, 4 * BC + 1), np.float32)
        xc[:, 0:BC] = dev[sl].T
        xc[:, BC : 2 * BC] = x1[sl].T
        xc[:, 2 * BC : 3 * BC] = x0[sl].T
        xc[:, 3 * BC : 4 * BC] = v[sl].T
        xc[:, 4 * BC] = tval
        in_maps.append(
            {"xc": xc, "aux": aux, "wp": wp, "kq": kq, "k2q": k2q, "w2t": w2t}
        )
    return in_maps


def kernel(t, state_batch, x0, x1, W1, b1, W2, b2):
    from concourse import bass_utils

    t = np.asarray(t)
    state_batch = np.asarray(state_batch, np.float32)
    x0 = np.asarray(x0, np.float32)
    x1 = np.asarray(x1, np.float32)
    W1 = np.asarray(W1, np.float32)
    b1 = np.asarray(b1, np.float32)
    W2 = np.asarray(W2, np.float32)

    nc = _get_program()
    in_maps = make_in_maps(t, state_batch, x0, x1, W1, b1, W2)
    res = bass_utils.run_bass_kernel_spmd(nc, in_maps, core_ids=list(range(NCORES)))
    acc = np.concatenate([res.results[c]["acc"] for c in range(NCORES)], axis=0)
    v = state_batch[B:]
    return np.concatenate([v, acc.astype(np.float32)], axis=0)
